# revision 31
# baseline (speedup 1.0000x reference)
"""Trainium2 Bass kernel for the SNN Leaky-Integrate-Fire problem.

Four-engine balanced scan. Per core: 8192 rows at [partition, chunk]
layout, state n = -mem, cn = -cur1, free dim = 64 chunks x 32 hidden =
2048 columns, 99 sequential steps of  m' = beta*m + cur1 - (m > 1).

Lanes (columns split by chunks; defaults A=24, B=28, F=12 chunks; every
tile has a single writer engine so the tile tracker never
false-synchronizes):
  A (DVE only, two interleaved half-lanes so consecutive DVE ops never
     wait on their immediate predecessor's write-ack):
       at = (n*-beta) - cn ; n' = (n is_lt -1) - at
  B (PE + DVE): PSUM = beta*I@n + I@cn (fp32 matmuls into two PSUM
     banks); n' = (n is_lt -1) + PSUM (one DVE op per bank)
  F (DVE+ACT+Pool): an = (n*beta) + cn [DVE]; h = Relu(Sign(-n-1))
     [ACT]; n' = h + an [Pool]. Halved only when CF > 12 so the
     Pool->ACT->Pool latency chain pipelines.

SBUF port discipline: an op whose src0/src1/dst fall in the same 256B
window (mod 1024B) runs up to 4x slower. The layout planner pads tile
bases so each hot op's APs land in distinct windows.

Numerics: DVE/ACT/Pool paths reproduce the reference's
fl(fl(beta*m)+c)-h roundings exactly; the PE lane's fl(beta*n) can
differ by 1ulp (double rounding), costing ~6 wrong spikes out of 2M --
rel err ~1.3e-3, far inside the 2e-2 gate.

cn = -cur1 is computed on DVE (6 TT ops with broadcast views of -W1
rows / -b1); fc2 = spike compare + per-o TT mult with broadcast W2 row
+ DVE tensor_reduce into ov (F products on Pool); host inverse-permutes
output rows.
"""
import os
import sys

sys.path.insert(0, "/opt/trn_rl_repo")

import numpy as np

import concourse.bacc as bacc
import concourse.tile as tile
from concourse import mybir
from concourse.bass_utils import run_bass_kernel_spmd

F32 = mybir.dt.float32
ALU = mybir.AluOpType
AF = mybir.ActivationFunctionType

# problem constants (hardcoded per contract)
B, N_IN, N_HID, N_OUT = 65536, 3, 32, 3
NUM_STEPS, BETA, THR = 100, 0.9, 1.0
N_CORES = 8
BC = B // N_CORES          # rows per core = 8192
P = 128                    # partitions
NCH = BC // P              # 128-row chunks per core = 64

# lane splits in chunks (32 cols each); all even so 256B classes stay fixed
CA = int(os.environ.get("KERNEL_CA", "26"))
CB = int(os.environ.get("KERNEL_CB", "28"))
CF = NCH - CA - CB
CF0 = CF if CF <= 12 else (CF // 2) & ~1
CF1 = CF - CF0
CB0 = min(16, (CB + 1) // 2)   # PSUM bank holds up to 512 fp32
CB1 = CB - CB0

# wb const layout [P, WB_COLS] (values replicated across partitions):
W1N_OFF = 0
B1N_OFF = 96
W2_OFF = 128
B2_OFF = 224
NO_OFF = 227
WB_COLS = 256


def build(nc, num_steps=NUM_STEPS):
    xA, xB, xF = CA * N_HID, CB * N_HID, CF * N_HID
    xB0 = CB0 * N_HID
    xF0, xF1 = CF0 * N_HID, CF1 * N_HID

    xk_d = nc.dram_tensor("xk", [P, N_IN * NCH], F32, kind="ExternalInput")
    wb_d = nc.dram_tensor("wb", [P, WB_COLS], F32, kind="ExternalInput")
    eye_d = nc.dram_tensor("eye", [P, 2 * P], F32, kind="ExternalInput")
    y_d = nc.dram_tensor("y", [BC, N_OUT], F32, kind="ExternalOutput")
    y_view = y_d[:].rearrange("(p i) o -> p (i o)", p=P)

    dve, gps, act = nc.vector, nc.gpsimd, nc.scalar

    # chunk ranges: A | F0 | F1 | B
    c_a, c_f0 = 0, CA
    c_f1 = CA + CF0
    c_b = CA + CF

    def h3(ap, cols):
        return ap.rearrange("p (i h) -> p i h", h=N_HID)

    with tile.TileContext(nc) as tc:
        with tc.tile_pool(name="pool", bufs=1) as pool, \
             tc.tile_pool(name="ps", bufs=1, space="PSUM") as psp:
            # ---- layout planner: sequential allocation with 256B classes
            state = {"off": 0, "pad": 0}

            def alloc(tag, cols, cls=None):
                if cls is not None:
                    need = (cls * 0x100 - state["off"]) % 0x400
                    if need:
                        assert need % 4 == 0
                        _pad = pool.tile([P, need // 4], F32,
                                         tag=f"pad{state['pad']}",
                                         name=f"pad{state['pad']}")
                        state["pad"] += 1
                        state["off"] += need
                t = pool.tile([P, cols], F32, tag=tag)
                state["off"] += cols * 4
                return t

            xk = alloc("xk", N_IN * NCH, 0)       # class 0 (192 cols)
            wt = alloc("wt", WB_COLS, 1)          # class 1
            eye = alloc("eye", 2 * P, 0)
            nc.sync.dma_start(xk[:], xk_d[:])
            nc.sync.dma_start(wt[:], wb_d[:])
            nc.sync.dma_start(eye[:], eye_d[:])

            nAF = CA + CF
            cn = alloc("cn", nAF * N_HID, 2)      # A∪F cn, class 2
            cnb = alloc("cnb", xB, 2) if CB else None
            sc = alloc("sc", NCH * N_HID, 3)      # cn scratch, class 3
            # lane A as two interleaved half-lanes (breaks same-engine
            # back-to-back RAW chains, which stall on own-sem write-acks)
            CA0 = (CA + 1) // 2
            xA0 = CA0 * N_HID
            xA1 = xA - xA0
            nt_a0 = alloc("nt_a0", xA0, 0) if CA else None
            at_a0 = alloc("at_a0", xA0, 1) if CA else None
            nt_a1 = alloc("nt_a1", xA1, 3) if xA1 else None
            at_a1 = alloc("at_a1", xA1, 0) if xA1 else None
            nt_b = alloc("nt_b", xB, 2) if CB else None
            sp_b = alloc("sp_b", xB, 3) if CB else None
            # F0: cn_f0 class 2 (CA even*128B). nt 0, an 1, ht 3
            nt_f0 = alloc("nt_f0", xF0, 0) if CF0 else None
            an_f0 = alloc("an_f0", xF0, 1) if CF0 else None
            ht_f0 = alloc("ht_f0", xF0, 3) if CF0 else None
            sc_f0 = alloc("sc_f0", xF0, 2) if CF0 else None
            bn_f0 = alloc("bn_f0", xF0, 3) if CF0 else None
            # F1: cn_f1 class = 2 + (CA+CF0)*0x80 -> computed below
            cf1_cls = (2 + ((c_f1 * 0x80) >> 8)) % 4
            pick = [c for c in (0, 1, 3, 2) if c != cf1_cls]
            nt_f1 = alloc("nt_f1", xF1, pick[0]) if CF1 else None
            an_f1 = alloc("an_f1", xF1, pick[1]) if CF1 else None
            ht_f1 = alloc("ht_f1", xF1, pick[2]) if CF1 else None
            sc_f1 = alloc("sc_f1", xF1, cf1_cls) if CF1 else None
            bn_f1 = alloc("bn_f1", xF1, pick[2]) if CF1 else None
            ov = alloc("ov", NCH * N_OUT, 0)
            CA0t = (CA + 1) // 2
            pr_a0 = alloc("pr_a0", CA0t * N_HID, 2) if CA else None
            pr_a1 = alloc("pr_a1", (CA - CA0t) * N_HID, 3) if CA - CA0t else None
            pr_b = alloc("pr_b", xB, 1) if CB else None

            xB1c = xB - xB0
            ps_a = psp.tile([P, xB0], F32, tag="ps_a", name="ps_a") if CB else None
            ps_bb = (psp.tile([P, xB1c], F32, tag="ps_bb", name="ps_bb")
                     if CB1 else None)

            def wbc(off, blocks):
                return (
                    wt[:, off: off + N_HID].unsqueeze(1)
                    .broadcast_to([P, blocks, N_HID])
                )

            def xbv(k, c0, nchk):
                return (
                    xk[:, k * NCH + c0: k * NCH + c0 + nchk].unsqueeze(2)
                    .broadcast_to([P, nchk, N_HID])
                )

            # ---- cn = -cur1 = sum_k x_k*(-W1[:,k]) + (-b1); A∪F sweep
            # first so those lanes' nt inits and scan overlap the B sweep
            def sweep(dst, c0, nchk, scoff):
                d3 = h3(dst, nchk * N_HID)
                s3 = h3(sc[:, scoff: scoff + nchk * N_HID], nchk * N_HID)
                dve.tensor_tensor(d3, xbv(0, c0, nchk), wbc(W1N_OFF, nchk),
                                  ALU.mult)
                dve.tensor_tensor(s3, xbv(1, c0, nchk),
                                  wbc(W1N_OFF + N_HID, nchk), ALU.mult)
                dve.tensor_tensor(d3, d3, s3, ALU.add)
                dve.tensor_tensor(s3, xbv(2, c0, nchk),
                                  wbc(W1N_OFF + 2 * N_HID, nchk), ALU.mult)
                dve.tensor_tensor(d3, d3, s3, ALU.add)
                dve.tensor_tensor(d3, d3, wbc(B1N_OFF, nchk), ALU.add)

            sweep(cn[:], 0, nAF, 0)
            if CB:
                sweep(cnb[:], nAF, CB, nAF * N_HID)

            cA0 = c_a * N_HID
            cF0o = c_f0 * N_HID
            cF1o = c_f1 * N_HID
            cBo = c_b * N_HID

            if CA:
                act.copy(nt_a0[:], cn[:, cA0: cA0 + xA0])
            if xA1:
                act.copy(nt_a1[:], cn[:, cA0 + xA0: cA0 + xA])
            if CF0:
                act.copy(nt_f0[:], cn[:, cF0o: cF0o + xF0])
            if CF1:
                act.copy(nt_f1[:], cn[:, cF1o: cF1o + xF1])
            if CB:
                act.copy(nt_b[:], cnb[:])

            cn_a0 = cn[:, cA0: cA0 + xA0]
            cn_a1 = cn[:, cA0 + xA0: cA0 + xA]
            cn_f0 = cn[:, cF0o: cF0o + xF0]
            cn_f1 = cn[:, cF1o: cF1o + xF1]

            cn_b0 = cnb[:, 0:xB0] if CB else None
            cn_b1 = cnb[:, xB0:xB] if CB1 else None

            negone = wt[:, NO_OFF: NO_OFF + 1]
            eyeB = eye[:, 0:P]
            eyeI = eye[:, P: 2 * P]

            # ---- scan steps 2..num_steps
            for _t in range(num_steps - 1):
                # PE lane B: PSUM = beta*I@n + I@cn per bank
                if CB:
                    nc.tensor.matmul(ps_a[:], eyeB, nt_b[:, 0:xB0],
                                     start=True, stop=False)
                    nc.tensor.matmul(ps_a[:], eyeI, cn_b0,
                                     start=False, stop=True)
                    if ps_bb is not None:
                        nc.tensor.matmul(ps_bb[:], eyeB, nt_b[:, xB0:xB],
                                         start=True, stop=False)
                        nc.tensor.matmul(ps_bb[:], eyeI, cn_b1,
                                         start=False, stop=True)
                # lane F halves
                if CF0:
                    dve.scalar_tensor_tensor(
                        an_f0[:], nt_f0[:], BETA, cn_f0, ALU.mult, ALU.add)
                    act.activation(ht_f0[:], nt_f0[:], AF.Sign,
                                   bias=negone, scale=-1.0)
                    act.activation(ht_f0[:], ht_f0[:], AF.Relu)
                    gps.tensor_tensor(nt_f0[:], ht_f0[:], an_f0[:], ALU.add)
                if CF1:
                    dve.scalar_tensor_tensor(
                        an_f1[:], nt_f1[:], BETA, cn_f1, ALU.mult, ALU.add)
                    act.activation(ht_f1[:], nt_f1[:], AF.Sign,
                                   bias=negone, scale=-1.0)
                    act.activation(ht_f1[:], ht_f1[:], AF.Relu)
                    gps.tensor_tensor(nt_f1[:], ht_f1[:], an_f1[:], ALU.add)
                # lane B consume; A affines slotted between the two banks
                # so DVE never idles waiting on the PE's second bank
                if CB:
                    dve.scalar_tensor_tensor(
                        nt_b[:, 0:xB0], nt_b[:, 0:xB0], -THR, ps_a[:],
                        ALU.is_lt, ALU.add)
                if CA:
                    dve.scalar_tensor_tensor(
                        at_a0[:], nt_a0[:], -BETA, cn_a0, ALU.mult, ALU.subtract)
                if xA1:
                    dve.scalar_tensor_tensor(
                        at_a1[:], nt_a1[:], -BETA, cn_a1, ALU.mult, ALU.subtract)
                if CB and ps_bb is not None:
                    dve.scalar_tensor_tensor(
                        nt_b[:, xB0:xB], nt_b[:, xB0:xB], -THR, ps_bb[:],
                        ALU.is_lt, ALU.add)
                if CA:
                    dve.scalar_tensor_tensor(
                        nt_a0[:], nt_a0[:], -THR, at_a0[:], ALU.is_lt, ALU.subtract)
                if xA1:
                    dve.scalar_tensor_tensor(
                        nt_a1[:], nt_a1[:], -THR, at_a1[:], ALU.is_lt, ALU.subtract)

            # ---- spikes (F on ACT; A/B spikes inside fc2 below)
            if CF0:
                act.activation(ht_f0[:], nt_f0[:], AF.Sign, bias=negone, scale=-1.0)
                act.activation(ht_f0[:], ht_f0[:], AF.Relu)
            if CF1:
                act.activation(ht_f1[:], nt_f1[:], AF.Sign, bias=negone, scale=-1.0)
                act.activation(ht_f1[:], ht_f1[:], AF.Relu)

            # ---- fc2: out[p, i, o] = sum_h spk * W2[o, h] (+ b2)
            # DVE lanes round-robin (A0, A1, B) with double-buffered product
            # tiles so no DVE op waits on its immediate predecessor's ack.
            ovv = ov[:].rearrange("p (i o) -> p o i", o=N_OUT)
            fprod = [(an_f0, nt_f0, sc_f0), (an_f1, nt_f1, sc_f1)]
            for hh, (half, xFh) in enumerate([(ht_f0, xF0), (ht_f1, xF1)]):
                if not xFh:
                    continue
                for o in range(N_OUT):
                    dst = fprod[hh][o]
                    gps.tensor_tensor(
                        h3(dst[:], xFh), h3(half[:], xFh),
                        wbc(W2_OFF + N_HID * o, xFh // N_HID), ALU.mult)

            lanes = []  # (spk_tile, prod0, prod1, ncols, chunk0, nch)
            if CA:
                lanes.append((at_a0, nt_a0, pr_a0, xA0, c_a, CA0))
            if xA1:
                lanes.append((at_a1, nt_a1, pr_a1, xA1, c_a + CA0,
                              CA - CA0))
            if CB:
                lanes.append((sp_b, nt_b, pr_b, xB, c_b, CB))
            if CA:
                dve.tensor_scalar(at_a0[:], nt_a0[:], -THR, None, ALU.is_lt)
            if xA1:
                dve.tensor_scalar(at_a1[:], nt_a1[:], -THR, None, ALU.is_lt)
            if CB:
                dve.tensor_scalar(sp_b[:], nt_b[:], -THR, None, ALU.is_lt)

            def pview(L, o):
                spk, p0, p1, xL, c0, nch = L
                return (p0[:] if (o & 1) == 0 else p1[:])

            for o in range(N_OUT):
                for L in lanes:
                    spk, p0, p1, xL, c0, nch = L
                    dve.tensor_tensor(
                        h3(pview(L, o), xL), h3(spk[:], xL),
                        wbc(W2_OFF + N_HID * o, nch), ALU.mult)
                for L in lanes:
                    spk, p0, p1, xL, c0, nch = L
                    dve.tensor_reduce(
                        ovv[:, o: o + 1, c0: c0 + nch], h3(pview(L, o), xL),
                        mybir.AxisListType.X, ALU.add)
            for hh, (c0, nchh) in enumerate([(c_f0, CF0), (c_f1, CF1)]):
                if not nchh:
                    continue
                for o in range(N_OUT):
                    srcp = fprod[hh][o]
                    dve.tensor_reduce(
                        ovv[:, o: o + 1, c0: c0 + nchh],
                        h3(srcp[:], nchh * N_HID), mybir.AxisListType.X, ALU.add)
            for o in range(N_OUT):
                dve.tensor_scalar(
                    ovv[:, o: o + 1, :], ovv[:, o: o + 1, :],
                    wt[:, B2_OFF + o: B2_OFF + o + 1], None, ALU.add)

            nc.sync.dma_start(y_view, ov[:])
    return nc


_CACHE = {}


def _get_program():
    if "nc" not in _CACHE:
        nc = bacc.Bacc("TRN2", target_bir_lowering=False, debug=False,
                       num_devices=N_CORES)
        build(nc)
        nc.compile()
        _CACHE["nc"] = nc
    return _CACHE["nc"]


def make_wb(b1, W2, b2):
    wb = np.zeros((P, WB_COLS), dtype=np.float32)
    wb[:, W2_OFF: W2_OFF + 3 * N_HID] = np.ascontiguousarray(W2).reshape(-1)
    wb[:, B2_OFF: B2_OFF + N_OUT] = b2
    wb[:, B1N_OFF: B1N_OFF + N_HID] = -b1
    wb[:, NO_OFF] = np.float32(-THR)
    return wb


def kernel(x, W1, b1, W2, b2):
    x = np.asarray(x, dtype=np.float32)
    W1, b1, W2, b2 = (np.asarray(a, dtype=np.float32) for a in (W1, b1, W2, b2))
    wb = make_wb(b1, W2, b2)
    for k in range(N_IN):
        wb[:, W1N_OFF + k * N_HID: W1N_OFF + (k + 1) * N_HID] = -W1[:, k]
    eye = np.zeros((P, 2 * P), dtype=np.float32)
    eye[np.arange(P), np.arange(P)] = np.float32(BETA)
    eye[np.arange(P), P + np.arange(P)] = np.float32(1.0)

    nc = _get_program()
    in_maps = []
    for i in range(N_CORES):
        xs = x[i * BC: (i + 1) * BC]              # [8192, 3]
        X3 = np.ascontiguousarray(xs.T).reshape(N_IN, NCH, P)
        xk = np.ascontiguousarray(X3.transpose(2, 0, 1)).reshape(P, N_IN * NCH)
        in_maps.append({"xk": xk, "wb": wb, "eye": eye})
    kwargs = dict(_CACHE.get("run_kwargs") or {})
    res = run_bass_kernel_spmd(nc, in_maps, core_ids=list(range(N_CORES)), **kwargs)
    _CACHE["last_results"] = res
    # y rows are stored permuted: dram row p*NCH + ch <-> logical row ch*P + p
    out = np.empty((B, N_OUT), dtype=np.float32)
    for i in range(N_CORES):
        yc = res.results[i]["y"].reshape(P, NCH, N_OUT)
        out[i * BC: (i + 1) * BC] = yc.transpose(1, 0, 2).reshape(BC, N_OUT)
    return out
